# revision 30
# baseline (speedup 1.0000x reference)
"""DenoiseGCN Trainium2 kernel.

Full-input contract: kernel(**inputs) takes the unsharded inputs from
setup_inputs() and returns the full (512, 2048) float32 output.

Strategy: pure data parallel over 8 NeuronCores (64 samples each, no
collectives). The per-core Bass/Tile kernel keeps each sample's
activations resident in SBUF in a feature-major layout
([features -> partitions, vertices -> free dim]) and runs the whole
network (time-MLP, 4 GCN layers with cycle aggregation + residuals,
MLP head) without touching HBM for intermediates.

Key kernel tricks:
  * cycle_agg commutes with the feature matmul:
      cycle_agg(h) @ W + h = (h[v-1]+h[v+1]) @ (W/3) + h @ (W/3 + I)
    so each layer is ONE shifted tensor_tensor add (vector/gpsimd,
    wraparound via 2-column halos on the h tiles) plus matmuls whose
    weights (W/3 and W/3+I) are prepared on the host.
  * matmuls run in float32r (full-rate fp32 mode: 1 cycle/row for
    512-wide moving operands); all matmul operand tiles are declared
    float32r so producers satisfy the fp32r-rounding BIR verifier.
  * the time embedding contributes a per-(sample, feature) constant in
    layer 0; it is computed on-device (temb @ (W0[2:]+res0_W[2:]) + b0)
    and applied through the scalar-engine activation bias port, so
    layer 0 needs only a K=4 matmul over [agg(coords); coords].
  * silu(psum + bias) is fused on the scalar engine reading PSUM
    directly; the head bias hb2 is folded into the PSUM drain as a
    per-partition tensor_scalar add on the vector engine.
  * emission is stage-interleaved across groups of G=4 samples so each
    engine's FIFO alternates between independent samples - without
    this the engines head-of-line block on the per-sample dependency
    chain (matmul -> silu -> aggregate -> matmul).

Measured on trn2 (8 cores): ~1.07 ms HW exec, absmax-relative error
~7.5e-4 vs the fp32 reference (float32r precision noise).
"""

import numpy as np

import concourse.bacc as bacc
import concourse.mybir as mybir
import concourse.tile as tile
from concourse.bass_utils import run_bass_kernel_spmd

F32 = mybir.dt.float32
F32R = mybir.dt.float32r
AF = mybir.ActivationFunctionType
ALU = mybir.AluOpType

NCORES = 8
B = 512
BPC = B // NCORES  # samples per core
V = 1024           # vertices per sample
HID = 256
TDIM = 128
DATA = 2048
HW = 1026          # haloed row width per feature-tile (1 + 1024 + 1)

def _sin_table():
    half = TDIM // 2
    freqs = np.exp(-np.log(10000.0) * np.arange(half, dtype=np.float64) / (half - 1))
    tt = np.arange(1000, dtype=np.float64)[:, None] * freqs[None, :]
    return np.concatenate([np.sin(tt), np.cos(tt)], axis=1).astype(np.float32)


_SIN_TABLE = _sin_table()

_PROG = None


def _build():
    nc = bacc.Bacc("TRN2", target_bir_lowering=False, debug=False, num_devices=NCORES)

    x = nc.dram_tensor("x", [2 * BPC, V], F32, kind="ExternalInput")
    embT = nc.dram_tensor("embT", [TDIM, BPC], F32, kind="ExternalInput")
    timeW = nc.dram_tensor("timeW", [TDIM, TDIM], F32, kind="ExternalInput")
    timeb = nc.dram_tensor("timeb", [TDIM], F32, kind="ExternalInput")
    w0cr = nc.dram_tensor("w0cr", [4, HID], F32, kind="ExternalInput")
    wsum = nc.dram_tensor("wsum", [TDIM, HID], F32, kind="ExternalInput")
    b0d = nc.dram_tensor("b0", [HID], F32, kind="ExternalInput")
    wld = [nc.dram_tensor(f"w{i}", [HID, HID], F32, kind="ExternalInput") for i in (1, 2, 3)]
    wid = [nc.dram_tensor(f"wi{i}", [HID, HID], F32, kind="ExternalInput") for i in (1, 2, 3)]
    bld = [nc.dram_tensor(f"b{i}", [HID], F32, kind="ExternalInput") for i in (1, 2, 3)]
    hw1 = nc.dram_tensor("hw1", [HID, HID], F32, kind="ExternalInput")
    hb1 = nc.dram_tensor("hb1", [HID], F32, kind="ExternalInput")
    hw2 = nc.dram_tensor("hw2", [HID, 2], F32, kind="ExternalInput")
    hb2 = nc.dram_tensor("hb2", [1, 2], F32, kind="ExternalInput")
    out = nc.dram_tensor("out", [2 * BPC, V], F32, kind="ExternalOutput")

    with tile.TileContext(nc) as tc:
        with (
            tc.tile_pool(name="const", bufs=1) as pc,
            tc.tile_pool(name="hp", bufs=8) as hp,
            tc.tile_pool(name="gp", bufs=5) as gp,
            tc.tile_pool(name="t4p", bufs=6) as t4p,
            tc.tile_pool(name="h5p", bufs=3) as h5p,
            tc.tile_pool(name="op", bufs=6) as op,
            tc.tile_pool(name="ps", bufs=6, space="PSUM") as ps,
            tc.tile_pool(name="ps2", bufs=2, space="PSUM") as ps2,
        ):
            dma = nc.sync.dma_start
            mm = nc.tensor.matmul
            act = nc.scalar.activation
            tt = nc.vector.tensor_tensor

            def ctile(shape, tag, src_ap=None, dt=F32):
                t = pc.tile(shape, dt, tag=tag)
                if src_ap is not None:
                    dma(t[:], src_ap.bitcast(dt) if dt is F32R else src_ap)
                return t

            cW0cR = ctile([4, HID], "cw0cr", w0cr[:], dt=F32R)
            cWsum = ctile([TDIM, HID], "cwsum", wsum[:], dt=F32R)
            cTW = ctile([TDIM, TDIM], "ctw", timeW[:], dt=F32R)
            ctb = ctile([TDIM, 1], "ctb", timeb[:].rearrange("(p o) -> p o", o=1))
            cb0 = ctile([128, 2], "cb0", b0d[:].rearrange("(m p) -> p m", m=2))
            cWl = [
                ctile([128, 2 * HID], f"cw{i}", wld[i][:].rearrange("(k p) m -> p k m", k=2), dt=F32R)
                for i in range(3)
            ]
            cWIl = [
                ctile([128, 2 * HID], f"cwi{i}", wid[i][:].rearrange("(k p) m -> p k m", k=2), dt=F32R)
                for i in range(3)
            ]
            cBl = [
                ctile([128, 2], f"cbl{i}", bld[i][:].rearrange("(m p) -> p m", m=2))
                for i in range(3)
            ]
            cHW1 = ctile([128, 2 * HID], "chw1", hw1[:].rearrange("(k p) m -> p k m", k=2), dt=F32R)
            cHB1 = ctile([128, 2], "chb1", hb1[:].rearrange("(m p) -> p m", m=2))
            cHW2 = ctile([128, 4], "chw2", hw2[:].rearrange("(k p) c -> p k c", k=2), dt=F32R)
            cHB2c = ctile([2, 1], "chb2c", hb2[:].rearrange("o p -> p o"))
            cEmb = ctile([TDIM, BPC], "cemb", embT[:], dt=F32R)
            # coords, feature-major batched over all samples:
            # partition p = 2*s + c holds x[s, c::2]
            cCoords = ctile([128, V], "ccoords", x[:], dt=F32R)

            # ---- time embedding MLP: temb = silu(emb @ time_W + time_b) ----
            pt = ps.tile([TDIM, BPC], F32, tag="ps")
            mm(pt[:], (cTW[:]), (cEmb[:]), start=True, stop=True)
            cTemb = ctile([TDIM, BPC], "ctemb", dt=F32R)
            act(cTemb[:], pt[:], AF.Silu, bias=ctb[:])

            # ---- per-sample layer-0 bias columns:
            # cb[:, m*BPC + s] = (temb_s @ (W0[2:]+res0_W[2:]) + b0)[m*128:(m+1)*128]
            cCB = ctile([128, 2 * BPC], "ccb")
            for m in range(2):
                pcb = ps.tile([128, BPC], F32, tag="ps")
                mm(pcb[:], (cWsum[:][:, m * 128:(m + 1) * 128]), (cTemb[:]),
                   start=True, stop=True)
                act(cCB[:][:, m * BPC:(m + 1) * BPC], pcb[:], AF.Identity, bias=cb0[:][:, m:m + 1])

            # ---- batched cycle-agg of coords (raw 3-term sum, no 1/3) ----
            cAggc = ctile([128, V], "caggc", dt=F32R)
            tt(cAggc[:][:, 1:1023], cCoords[:][:, 0:1022], cCoords[:][:, 2:1024], ALU.add)
            tt(cAggc[:][:, 0:1], cCoords[:][:, 1023:1024], cCoords[:][:, 1:2], ALU.add)
            tt(cAggc[:][:, 1023:1024], cCoords[:][:, 1022:1023], cCoords[:][:, 0:1], ALU.add)
            tt(cAggc[:], cAggc[:], cCoords[:], ALU.add)

            # ---- main pipeline, stage-interleaved across groups of G samples
            # so each engine's FIFO alternates samples (no head-of-line
            # blocking while another sample's aggregation runs).
            st = {}  # per-sample pipeline state: current h tile

            def stage_t4(s):
                t4 = t4p.tile([4, V], F32R, tag="t4", name="t4")
                dma(t4[0:2, :], cAggc[2 * s:2 * s + 2, :])
                dma(t4[2:4, :], cCoords[2 * s:2 * s + 2, :])
                st[s] = {"t4": t4}

            def stage_l0(s):
                # layer 0: h1 = silu(aggc@W0c/3 + coords@res0c + cb_s)
                t4 = st[s].pop("t4")
                h = hp.tile([128, 2 * HW], F32R, tag="h", name="h")
                h3 = h[:].rearrange("p (m v) -> p m v", m=2)
                for m in range(2):
                    for c in range(2):
                        p = ps.tile([128, 512], F32, tag="ps", name="pc")
                        mm(p[:],
                           (cW0cR[:][:, m * 128:(m + 1) * 128]),
                           (t4[:][:, c * 512:(c + 1) * 512]),
                           start=True, stop=True)
                        act(h[:][:, m * HW + 1 + c * 512:m * HW + 1 + (c + 1) * 512],
                            p[:], AF.Silu,
                            bias=cCB[:][:, m * BPC + s:m * BPC + s + 1])
                nc.vector.tensor_copy(h3[:, :, 0:1], h3[:, :, 1024:1025])
                nc.vector.tensor_copy(h3[:, :, 1025:1026], h3[:, :, 1:2])
                st[s]["h"] = (h, h3)

            def stage_layer(s, li):
                # h <- silu(cycle_agg(h)@(W/3) + h + b)
                h, h3 = st[s]["h"]
                cW = cWl[li]
                cWI = cWIl[li]
                cB = cBl[li]
                eng = nc.gpsimd if (3 * s + li) % 2 == 0 else nc.vector
                g = gp.tile([128, 2 * V], F32R, tag="g", name="g")
                g3 = g[:].rearrange("p (m v) -> p m v", m=2)
                eng.tensor_tensor(g3, h3[:, :, 0:1024], h3[:, :, 2:1026], ALU.add)
                hn = hp.tile([128, 2 * HW], F32R, tag="h", name="h")
                hn3 = hn[:].rearrange("p (m v) -> p m v", m=2)
                for m in range(2):
                    pcs = [ps.tile([128, 512], F32, tag="ps", name="pc") for _ in range(2)]
                    for k in range(2):
                        for c in range(2):
                            mm(pcs[c][:], (cW[:][:, k * HID + m * 128:k * HID + (m + 1) * 128]),
                               (g[:][:, k * V + c * 512:k * V + (c + 1) * 512]),
                               start=(k == 0), stop=False)
                    for k in range(2):
                        for c in range(2):
                            last = k == 1
                            mm(pcs[c][:],
                               (cWI[:][:, k * HID + m * 128:k * HID + (m + 1) * 128]),
                               (h[:][:, k * HW + 1 + c * 512:k * HW + 1 + (c + 1) * 512]),
                               start=False, stop=last)
                            if last:
                                act(hn[:][:, m * HW + 1 + c * 512:m * HW + 1 + (c + 1) * 512],
                                    pcs[c][:], AF.Silu, bias=cB[:][:, m:m + 1])
                nc.vector.tensor_copy(hn3[:, :, 0:1], hn3[:, :, 1024:1025])
                nc.vector.tensor_copy(hn3[:, :, 1025:1026], hn3[:, :, 1:2])
                st[s]["h"] = (hn, hn3)

            def stage_m1(s):
                h, _ = st[s].pop("h")
                h5 = h5p.tile([128, 2 * V], F32R, tag="h5", name="h5")
                for m in range(2):
                    pcs = [ps.tile([128, 512], F32, tag="ps", name="pc") for _ in range(2)]
                    for c in range(2):
                        mm(pcs[c][:], (cHW1[:][:, m * 128:(m + 1) * 128]),
                           (h[:][:, 1 + c * 512:1 + (c + 1) * 512]),
                           start=True, stop=False)
                    for c in range(2):
                        mm(pcs[c][:], (cHW1[:][:, HID + m * 128:HID + (m + 1) * 128]),
                           (h[:][:, HW + 1 + c * 512:HW + 1 + (c + 1) * 512]),
                           start=False, stop=True)
                        act(h5[:][:, m * V + c * 512:m * V + (c + 1) * 512],
                            pcs[c][:], AF.Silu, bias=cHB1[:][:, m:m + 1])
                st[s]["h5"] = h5

            def stage_m2(s):
                h5 = st[s].pop("h5")
                osb = op.tile([2, V], F32, tag="osb", name="osb")
                for c in range(2):
                    pm2 = ps2.tile([2, 512], F32, tag="ps2", name="pm2")
                    mm(pm2[:], (cHW2[:][:, 0:2]), (h5[:][:, c * 512:(c + 1) * 512]),
                       start=True, stop=False)
                    mm(pm2[:], (cHW2[:][:, 2:4]), (h5[:][:, V + c * 512:V + (c + 1) * 512]),
                       start=False, stop=True)
                    nc.vector.tensor_scalar_add(osb[:][:, c * 512:(c + 1) * 512],
                                                pm2[:], cHB2c[:])
                dma(out[2 * s:2 * s + 2, :], osb[:])

            G = 4
            stages = ([stage_t4, stage_l0]
                      + [lambda s, li=li: stage_layer(s, li) for li in range(3)]
                      + [stage_m1, stage_m2])
            for s0 in range(0, BPC, G):
                group = range(s0, min(s0 + G, BPC))
                for fn in stages:
                    for s in group:
                        fn(s)

    nc.compile()
    return nc


def _get_prog():
    global _PROG
    if _PROG is None:
        _PROG = _build()
    return _PROG


def kernel(**inputs) -> np.ndarray:
    f = lambda a: np.ascontiguousarray(np.asarray(a, dtype=np.float32))
    x = f(inputs["x"])
    t = np.asarray(inputs["t"]).astype(np.int64)
    time_W = f(inputs["time_W"])
    time_b = f(inputs["time_b"])
    W0, b0 = f(inputs["W0"]), f(inputs["b0"])
    Ws = [f(inputs[k]) for k in ("W1", "W2", "W3")]
    bs = [f(inputs[k]) for k in ("b1", "b2", "b3")]
    res0_W = f(inputs["res0_W"])
    hW1, hb1 = f(inputs["hW1"]), f(inputs["hb1"])
    hW2, hb2 = f(inputs["hW2"]), f(inputs["hb2"])

    emb = _SIN_TABLE[t]  # (B, TDIM) gather from the constant sinusoid table

    shared = {
        "timeW": time_W,
        "timeb": time_b,
        "w0cr": np.concatenate([W0[:2] / 3.0, res0_W[:2]], axis=0),
        "wsum": W0[2:] + res0_W[2:],
        "b0": b0,
        "hw1": hW1,
        "hb1": hb1,
        "hw2": hW2,
        "hb2": hb2.reshape(1, 2),
    }
    eye256 = np.eye(HID, dtype=np.float32)
    for i in range(3):
        shared[f"w{i + 1}"] = np.ascontiguousarray(Ws[i] / 3.0)
        shared[f"wi{i + 1}"] = np.ascontiguousarray(Ws[i] / 3.0 + eye256)
        shared[f"b{i + 1}"] = bs[i]

    in_maps = []
    for c in range(NCORES):
        sl = slice(c * BPC, (c + 1) * BPC)
        m = dict(shared)
        # (BPC, 2048) -> (BPC, V, 2) -> (BPC, 2, V) -> (2*BPC, V): row 2s+c = x[s, c::2]
        m["x"] = np.ascontiguousarray(
            x[sl].reshape(BPC, V, 2).transpose(0, 2, 1).reshape(2 * BPC, V))
        m["embT"] = np.ascontiguousarray(emb[sl].T)
        in_maps.append(m)

    nc = _get_prog()
    res = run_bass_kernel_spmd(nc, in_maps, list(range(NCORES)))
    outs = []
    for i in range(NCORES):
        o = res.results[i]["out"]  # (2*BPC, V), row 2s+c = out[s, c::2]
        outs.append(o.reshape(BPC, 2, V).transpose(0, 2, 1).reshape(BPC, DATA))
    return np.concatenate(outs, axis=0)


if __name__ == "__main__":
    rng = np.random.default_rng(0)
    demo = {
        "x": rng.standard_normal((B, DATA), dtype=np.float32),
        "t": rng.integers(0, 1000, size=(B,)).astype(np.int32),
        "time_W": rng.standard_normal((TDIM, TDIM), dtype=np.float32) / 11.3,
        "time_b": np.zeros(TDIM, np.float32),
        "W0": rng.standard_normal((130, HID), dtype=np.float32) / 11.4,
        "b0": np.zeros(HID, np.float32),
        "W1": rng.standard_normal((HID, HID), dtype=np.float32) / 16.0,
        "b1": np.zeros(HID, np.float32),
        "W2": rng.standard_normal((HID, HID), dtype=np.float32) / 16.0,
        "b2": np.zeros(HID, np.float32),
        "W3": rng.standard_normal((HID, HID), dtype=np.float32) / 16.0,
        "b3": np.zeros(HID, np.float32),
        "res0_W": rng.standard_normal((130, HID), dtype=np.float32) / 11.4,
        "hW1": rng.standard_normal((HID, HID), dtype=np.float32) / 16.0,
        "hb1": np.zeros(HID, np.float32),
        "hW2": rng.standard_normal((HID, 2), dtype=np.float32) / 16.0,
        "hb2": np.zeros(2, np.float32),
    }
    out = kernel(**demo)
    print("out", out.shape, out.dtype, float(np.abs(out).mean()))


# revision 31
# speedup vs baseline: 1.0059x; 1.0059x over previous
"""DenoiseGCN Trainium2 kernel.

Full-input contract: kernel(**inputs) takes the unsharded inputs from
setup_inputs() and returns the full (512, 2048) float32 output.

Strategy: pure data parallel over 8 NeuronCores (64 samples each, no
collectives). The per-core Bass/Tile kernel keeps each sample's
activations resident in SBUF in a feature-major layout
([features -> partitions, vertices -> free dim]) and runs the whole
network (time-MLP, 4 GCN layers with cycle aggregation + residuals,
MLP head) without touching HBM for intermediates.

Key kernel tricks:
  * cycle_agg commutes with the feature matmul:
      cycle_agg(h) @ W + h = (h[v-1]+h[v+1]) @ (W/3) + h @ (W/3 + I)
    so each layer is ONE shifted tensor_tensor add (vector/gpsimd,
    wraparound via 2-column halos on the h tiles) plus matmuls whose
    weights (W/3 and W/3+I) are prepared on the host.
  * matmuls run in float32r (full-rate fp32 mode: 1 cycle/row for
    512-wide moving operands); all matmul operand tiles are declared
    float32r so producers satisfy the fp32r-rounding BIR verifier.
  * the time embedding contributes a per-(sample, feature) constant in
    layer 0; it is computed on-device (temb @ (W0[2:]+res0_W[2:]) + b0)
    and applied through the scalar-engine activation bias port, so
    layer 0 needs only a K=4 matmul over [agg(coords); coords].
  * silu(psum + bias) is fused on the scalar engine reading PSUM
    directly; the head bias hb2 is folded into the PSUM drain as a
    per-partition tensor_scalar add on the vector engine.
  * emission is stage-interleaved across groups of G=4 samples so each
    engine's FIFO alternates between independent samples - without
    this the engines head-of-line block on the per-sample dependency
    chain (matmul -> silu -> aggregate -> matmul).

Measured on trn2 (8 cores): ~1.07 ms HW exec, absmax-relative error
~7.5e-4 vs the fp32 reference (float32r precision noise).
"""

import numpy as np

import concourse.bacc as bacc
import concourse.mybir as mybir
import concourse.tile as tile
from concourse.bass_utils import run_bass_kernel_spmd

F32 = mybir.dt.float32
F32R = mybir.dt.float32r
AF = mybir.ActivationFunctionType
ALU = mybir.AluOpType

NCORES = 8
B = 512
BPC = B // NCORES  # samples per core
V = 1024           # vertices per sample
HID = 256
TDIM = 128
DATA = 2048
HW = 1026          # haloed row width per feature-tile (1 + 1024 + 1)

def _sin_table():
    half = TDIM // 2
    freqs = np.exp(-np.log(10000.0) * np.arange(half, dtype=np.float64) / (half - 1))
    tt = np.arange(1000, dtype=np.float64)[:, None] * freqs[None, :]
    return np.concatenate([np.sin(tt), np.cos(tt)], axis=1).astype(np.float32)


_SIN_TABLE = _sin_table()

_PROG = None


def _build():
    nc = bacc.Bacc("TRN2", target_bir_lowering=False, debug=False, num_devices=NCORES)

    x = nc.dram_tensor("x", [2 * BPC, V], F32, kind="ExternalInput")
    embT = nc.dram_tensor("embT", [TDIM, BPC], F32, kind="ExternalInput")
    timeW = nc.dram_tensor("timeW", [TDIM, TDIM], F32, kind="ExternalInput")
    timeb = nc.dram_tensor("timeb", [TDIM], F32, kind="ExternalInput")
    w0cr = nc.dram_tensor("w0cr", [4, HID], F32, kind="ExternalInput")
    wsum = nc.dram_tensor("wsum", [TDIM, HID], F32, kind="ExternalInput")
    b0d = nc.dram_tensor("b0", [HID], F32, kind="ExternalInput")
    wld = [nc.dram_tensor(f"w{i}", [HID, HID], F32, kind="ExternalInput") for i in (1, 2, 3)]
    wid = [nc.dram_tensor(f"wi{i}", [HID, HID], F32, kind="ExternalInput") for i in (1, 2, 3)]
    bld = [nc.dram_tensor(f"b{i}", [HID], F32, kind="ExternalInput") for i in (1, 2, 3)]
    hw1 = nc.dram_tensor("hw1", [HID, HID], F32, kind="ExternalInput")
    hb1 = nc.dram_tensor("hb1", [HID], F32, kind="ExternalInput")
    hw2 = nc.dram_tensor("hw2", [HID, 2], F32, kind="ExternalInput")
    hb2 = nc.dram_tensor("hb2", [1, 2], F32, kind="ExternalInput")
    out = nc.dram_tensor("out", [2 * BPC, V], F32, kind="ExternalOutput")

    with tile.TileContext(nc) as tc:
        with (
            tc.tile_pool(name="const", bufs=1) as pc,
            tc.tile_pool(name="hp", bufs=8) as hp,
            tc.tile_pool(name="gp", bufs=5) as gp,
            tc.tile_pool(name="t4p", bufs=6) as t4p,
            tc.tile_pool(name="h5p", bufs=3) as h5p,
            tc.tile_pool(name="op", bufs=6) as op,
            tc.tile_pool(name="ps", bufs=6, space="PSUM") as ps,
            tc.tile_pool(name="ps2", bufs=2, space="PSUM") as ps2,
        ):
            dma = nc.sync.dma_start
            mm = nc.tensor.matmul
            act = nc.scalar.activation
            tt = nc.vector.tensor_tensor

            def ctile(shape, tag, src_ap=None, dt=F32):
                t = pc.tile(shape, dt, tag=tag)
                if src_ap is not None:
                    dma(t[:], src_ap.bitcast(dt) if dt is F32R else src_ap)
                return t

            # critical-path constants first: coords + everything the
            # time-MLP / layer-0 chain needs, so sample 0's matmuls start
            # as early as possible; bulk layer/head weights load after.
            cCoords = ctile([128, V], "ccoords", x[:], dt=F32R)
            cEmb = ctile([TDIM, BPC], "cemb", embT[:], dt=F32R)
            cTW = ctile([TDIM, TDIM], "ctw", timeW[:], dt=F32R)
            ctb = ctile([TDIM, 1], "ctb", timeb[:].rearrange("(p o) -> p o", o=1))
            cWsum = ctile([TDIM, HID], "cwsum", wsum[:], dt=F32R)
            cb0 = ctile([128, 2], "cb0", b0d[:].rearrange("(m p) -> p m", m=2))
            cW0cR = ctile([4, HID], "cw0cr", w0cr[:], dt=F32R)
            cWl = [
                ctile([128, 2 * HID], f"cw{i}", wld[i][:].rearrange("(k p) m -> p k m", k=2), dt=F32R)
                for i in range(3)
            ]
            cWIl = [
                ctile([128, 2 * HID], f"cwi{i}", wid[i][:].rearrange("(k p) m -> p k m", k=2), dt=F32R)
                for i in range(3)
            ]
            cBl = [
                ctile([128, 2], f"cbl{i}", bld[i][:].rearrange("(m p) -> p m", m=2))
                for i in range(3)
            ]
            cHW1 = ctile([128, 2 * HID], "chw1", hw1[:].rearrange("(k p) m -> p k m", k=2), dt=F32R)
            cHB1 = ctile([128, 2], "chb1", hb1[:].rearrange("(m p) -> p m", m=2))
            cHW2 = ctile([128, 4], "chw2", hw2[:].rearrange("(k p) c -> p k c", k=2), dt=F32R)
            cHB2c = ctile([2, 1], "chb2c", hb2[:].rearrange("o p -> p o"))

            # ---- time embedding MLP: temb = silu(emb @ time_W + time_b) ----
            pt = ps.tile([TDIM, BPC], F32, tag="ps")
            mm(pt[:], (cTW[:]), (cEmb[:]), start=True, stop=True)
            cTemb = ctile([TDIM, BPC], "ctemb", dt=F32R)
            act(cTemb[:], pt[:], AF.Silu, bias=ctb[:])

            # ---- per-sample layer-0 bias columns:
            # cb[:, m*BPC + s] = (temb_s @ (W0[2:]+res0_W[2:]) + b0)[m*128:(m+1)*128]
            cCB = ctile([128, 2 * BPC], "ccb")
            for m in range(2):
                pcb = ps.tile([128, BPC], F32, tag="ps")
                mm(pcb[:], (cWsum[:][:, m * 128:(m + 1) * 128]), (cTemb[:]),
                   start=True, stop=True)
                act(cCB[:][:, m * BPC:(m + 1) * BPC], pcb[:], AF.Identity, bias=cb0[:][:, m:m + 1])

            # ---- batched cycle-agg of coords (raw 3-term sum, no 1/3) ----
            cAggc = ctile([128, V], "caggc", dt=F32R)
            tt(cAggc[:][:, 1:1023], cCoords[:][:, 0:1022], cCoords[:][:, 2:1024], ALU.add)
            tt(cAggc[:][:, 0:1], cCoords[:][:, 1023:1024], cCoords[:][:, 1:2], ALU.add)
            tt(cAggc[:][:, 1023:1024], cCoords[:][:, 1022:1023], cCoords[:][:, 0:1], ALU.add)
            tt(cAggc[:], cAggc[:], cCoords[:], ALU.add)

            # ---- main pipeline, stage-interleaved across groups of G samples
            # so each engine's FIFO alternates samples (no head-of-line
            # blocking while another sample's aggregation runs).
            st = {}  # per-sample pipeline state: current h tile

            def stage_t4(s):
                t4 = t4p.tile([4, V], F32R, tag="t4", name="t4")
                dma(t4[0:2, :], cAggc[2 * s:2 * s + 2, :])
                dma(t4[2:4, :], cCoords[2 * s:2 * s + 2, :])
                st[s] = {"t4": t4}

            def stage_l0(s):
                # layer 0: h1 = silu(aggc@W0c/3 + coords@res0c + cb_s)
                t4 = st[s].pop("t4")
                h = hp.tile([128, 2 * HW], F32R, tag="h", name="h")
                h3 = h[:].rearrange("p (m v) -> p m v", m=2)
                for m in range(2):
                    for c in range(2):
                        p = ps.tile([128, 512], F32, tag="ps", name="pc")
                        mm(p[:],
                           (cW0cR[:][:, m * 128:(m + 1) * 128]),
                           (t4[:][:, c * 512:(c + 1) * 512]),
                           start=True, stop=True)
                        act(h[:][:, m * HW + 1 + c * 512:m * HW + 1 + (c + 1) * 512],
                            p[:], AF.Silu,
                            bias=cCB[:][:, m * BPC + s:m * BPC + s + 1])
                nc.vector.tensor_copy(h3[:, :, 0:1], h3[:, :, 1024:1025])
                nc.vector.tensor_copy(h3[:, :, 1025:1026], h3[:, :, 1:2])
                st[s]["h"] = (h, h3)

            def stage_layer(s, li):
                # h <- silu(cycle_agg(h)@(W/3) + h + b)
                h, h3 = st[s]["h"]
                cW = cWl[li]
                cWI = cWIl[li]
                cB = cBl[li]
                eng = nc.gpsimd if (3 * s + li) % 2 == 0 else nc.vector
                g = gp.tile([128, 2 * V], F32R, tag="g", name="g")
                g3 = g[:].rearrange("p (m v) -> p m v", m=2)
                eng.tensor_tensor(g3, h3[:, :, 0:1024], h3[:, :, 2:1026], ALU.add)
                hn = hp.tile([128, 2 * HW], F32R, tag="h", name="h")
                hn3 = hn[:].rearrange("p (m v) -> p m v", m=2)
                for m in range(2):
                    pcs = [ps.tile([128, 512], F32, tag="ps", name="pc") for _ in range(2)]
                    for k in range(2):
                        for c in range(2):
                            mm(pcs[c][:], (cW[:][:, k * HID + m * 128:k * HID + (m + 1) * 128]),
                               (g[:][:, k * V + c * 512:k * V + (c + 1) * 512]),
                               start=(k == 0), stop=False)
                    for k in range(2):
                        for c in range(2):
                            last = k == 1
                            mm(pcs[c][:],
                               (cWI[:][:, k * HID + m * 128:k * HID + (m + 1) * 128]),
                               (h[:][:, k * HW + 1 + c * 512:k * HW + 1 + (c + 1) * 512]),
                               start=False, stop=last)
                            if last:
                                act(hn[:][:, m * HW + 1 + c * 512:m * HW + 1 + (c + 1) * 512],
                                    pcs[c][:], AF.Silu, bias=cB[:][:, m:m + 1])
                nc.vector.tensor_copy(hn3[:, :, 0:1], hn3[:, :, 1024:1025])
                nc.vector.tensor_copy(hn3[:, :, 1025:1026], hn3[:, :, 1:2])
                st[s]["h"] = (hn, hn3)

            def stage_m1(s):
                h, _ = st[s].pop("h")
                h5 = h5p.tile([128, 2 * V], F32R, tag="h5", name="h5")
                for m in range(2):
                    pcs = [ps.tile([128, 512], F32, tag="ps", name="pc") for _ in range(2)]
                    for c in range(2):
                        mm(pcs[c][:], (cHW1[:][:, m * 128:(m + 1) * 128]),
                           (h[:][:, 1 + c * 512:1 + (c + 1) * 512]),
                           start=True, stop=False)
                    for c in range(2):
                        mm(pcs[c][:], (cHW1[:][:, HID + m * 128:HID + (m + 1) * 128]),
                           (h[:][:, HW + 1 + c * 512:HW + 1 + (c + 1) * 512]),
                           start=False, stop=True)
                        act(h5[:][:, m * V + c * 512:m * V + (c + 1) * 512],
                            pcs[c][:], AF.Silu, bias=cHB1[:][:, m:m + 1])
                st[s]["h5"] = h5

            def stage_m2(s):
                h5 = st[s].pop("h5")
                osb = op.tile([2, V], F32, tag="osb", name="osb")
                for c in range(2):
                    pm2 = ps2.tile([2, 512], F32, tag="ps2", name="pm2")
                    mm(pm2[:], (cHW2[:][:, 0:2]), (h5[:][:, c * 512:(c + 1) * 512]),
                       start=True, stop=False)
                    mm(pm2[:], (cHW2[:][:, 2:4]), (h5[:][:, V + c * 512:V + (c + 1) * 512]),
                       start=False, stop=True)
                    nc.vector.tensor_scalar_add(osb[:][:, c * 512:(c + 1) * 512],
                                                pm2[:], cHB2c[:])
                dma(out[2 * s:2 * s + 2, :], osb[:])

            G = 4
            stages = ([stage_t4, stage_l0]
                      + [lambda s, li=li: stage_layer(s, li) for li in range(3)]
                      + [stage_m1, stage_m2])
            for s0 in range(0, BPC, G):
                group = range(s0, min(s0 + G, BPC))
                for fn in stages:
                    for s in group:
                        fn(s)

    nc.compile()
    return nc


def _get_prog():
    global _PROG
    if _PROG is None:
        _PROG = _build()
    return _PROG


def kernel(**inputs) -> np.ndarray:
    f = lambda a: np.ascontiguousarray(np.asarray(a, dtype=np.float32))
    x = f(inputs["x"])
    t = np.asarray(inputs["t"]).astype(np.int64)
    time_W = f(inputs["time_W"])
    time_b = f(inputs["time_b"])
    W0, b0 = f(inputs["W0"]), f(inputs["b0"])
    Ws = [f(inputs[k]) for k in ("W1", "W2", "W3")]
    bs = [f(inputs[k]) for k in ("b1", "b2", "b3")]
    res0_W = f(inputs["res0_W"])
    hW1, hb1 = f(inputs["hW1"]), f(inputs["hb1"])
    hW2, hb2 = f(inputs["hW2"]), f(inputs["hb2"])

    emb = _SIN_TABLE[t]  # (B, TDIM) gather from the constant sinusoid table

    shared = {
        "timeW": time_W,
        "timeb": time_b,
        "w0cr": np.concatenate([W0[:2] / 3.0, res0_W[:2]], axis=0),
        "wsum": W0[2:] + res0_W[2:],
        "b0": b0,
        "hw1": hW1,
        "hb1": hb1,
        "hw2": hW2,
        "hb2": hb2.reshape(1, 2),
    }
    eye256 = np.eye(HID, dtype=np.float32)
    for i in range(3):
        shared[f"w{i + 1}"] = np.ascontiguousarray(Ws[i] / 3.0)
        shared[f"wi{i + 1}"] = np.ascontiguousarray(Ws[i] / 3.0 + eye256)
        shared[f"b{i + 1}"] = bs[i]

    in_maps = []
    for c in range(NCORES):
        sl = slice(c * BPC, (c + 1) * BPC)
        m = dict(shared)
        # (BPC, 2048) -> (BPC, V, 2) -> (BPC, 2, V) -> (2*BPC, V): row 2s+c = x[s, c::2]
        m["x"] = np.ascontiguousarray(
            x[sl].reshape(BPC, V, 2).transpose(0, 2, 1).reshape(2 * BPC, V))
        m["embT"] = np.ascontiguousarray(emb[sl].T)
        in_maps.append(m)

    nc = _get_prog()
    res = run_bass_kernel_spmd(nc, in_maps, list(range(NCORES)))
    outs = []
    for i in range(NCORES):
        o = res.results[i]["out"]  # (2*BPC, V), row 2s+c = out[s, c::2]
        outs.append(o.reshape(BPC, 2, V).transpose(0, 2, 1).reshape(BPC, DATA))
    return np.concatenate(outs, axis=0)


if __name__ == "__main__":
    rng = np.random.default_rng(0)
    demo = {
        "x": rng.standard_normal((B, DATA), dtype=np.float32),
        "t": rng.integers(0, 1000, size=(B,)).astype(np.int32),
        "time_W": rng.standard_normal((TDIM, TDIM), dtype=np.float32) / 11.3,
        "time_b": np.zeros(TDIM, np.float32),
        "W0": rng.standard_normal((130, HID), dtype=np.float32) / 11.4,
        "b0": np.zeros(HID, np.float32),
        "W1": rng.standard_normal((HID, HID), dtype=np.float32) / 16.0,
        "b1": np.zeros(HID, np.float32),
        "W2": rng.standard_normal((HID, HID), dtype=np.float32) / 16.0,
        "b2": np.zeros(HID, np.float32),
        "W3": rng.standard_normal((HID, HID), dtype=np.float32) / 16.0,
        "b3": np.zeros(HID, np.float32),
        "res0_W": rng.standard_normal((130, HID), dtype=np.float32) / 11.4,
        "hW1": rng.standard_normal((HID, HID), dtype=np.float32) / 16.0,
        "hb1": np.zeros(HID, np.float32),
        "hW2": rng.standard_normal((HID, 2), dtype=np.float32) / 16.0,
        "hb2": np.zeros(2, np.float32),
    }
    out = kernel(**demo)
    print("out", out.shape, out.dtype, float(np.abs(out).mean()))
